# revision 15
# baseline (speedup 1.0000x reference)
"""MultiHeadSelfAttention Trainium2 Bass kernel (v4).

Shapes (hardcoded): B=8, N=2048, E=512, H=8 heads, D=64 head dim.
Sharding: data-parallel over batch -> one batch item per NeuronCore (8 cores),
no collectives needed.

v4 design (vs v2 at ~420us measured):
  - Inputs are cast to bf16 on the HOST (the kernel computed in bf16 SBUF
    tiles anyway), so ALL stage-0 transposes (Q/K/V/W -> column-major) run
    as DMA xbar transposes straight from DRAM -- one [2048,128]->[128,2048]
    instruction per (input, e-tile): zero PE/DVE/ACT cost where v2 spent
    78us PE (fp32 transposes) + 84us DVE + 26us ACT (copy casts).
  - Output projection runs in direct (non-transposed) orientation:
    y[n-tile] = sum_et oTn[et][:, n-block]^T @ WoT[et] -- the attention
    output O^T is the lhsT, so no de-transpose/cast tail at all. Units are
    dripped late (kt>=9) into hp3's chunks so the in-order PE stream never
    blocks on the per-chunk norm DMA chain.
  - Normalization is batched per-hp for hp0-2 (one [128,2048] multiply +
    6 DMAs instead of 4x per-qc chains); hp3 keeps per-qc norm so y units
    unlock chunk by chunk.
  - exp offload history: DVE tensor_tensor pow fails the TRN2 ISA engine
    check; GpSimd pow compiles but runs as a DSP software loop (~169us per
    [128,1024] tile, measured) -- both dead ends. All 256 exps stay on
    ScalarE (the kernel's ~294us floor); everything else hides under them.
    Scores arrive pre-scaled by log2e/tau (folded into Wq on the host), so
    the ACTIVATE uses scale=LN2.
  - Attention core unchanged from v2: scores 2 heads/kt via PE row tiling,
    one [128,1024] exp per kt, PV lagged one kt, v_aug ones-column gives
    softmax denominators for free.
"""

import sys

for _p in ("/opt/trn_rl_repo",):
    if _p not in sys.path:
        sys.path.insert(0, _p)

import numpy as np
from collections import deque
from contextlib import ExitStack

import concourse.bass as bass
import concourse.bacc as bacc
import concourse.mybir as mybir
import concourse.tile as tile

B, N, E = 8, 2048, 512
H, D = 8, 64
P = 128          # partitions
ET = E // P      # 4 e-tiles
NT = N // P      # 16 n-tiles
QC = 512         # q chunk in attention
NQC = N // QC    # 4
HV = 65          # head dim + ones column
FP32 = mybir.dt.float32
BF16 = mybir.dt.bfloat16
NCORES = 8

AF = mybir.ActivationFunctionType
ALU = mybir.AluOpType
LOG2E = 1.4426950408889634
LN2 = 0.6931471805599453


def _build() -> bass.Bass:
    nc = bacc.Bacc(trn_type="TRN2")

    dQ = nc.dram_tensor("Q", [N, E], BF16, kind="ExternalInput")
    dK = nc.dram_tensor("K", [N, E], BF16, kind="ExternalInput")
    dV = nc.dram_tensor("V", [N, E], BF16, kind="ExternalInput")
    dW = {
        "q": nc.dram_tensor("Wq", [E, E], BF16, kind="ExternalInput"),
        "k": nc.dram_tensor("Wk", [E, E], BF16, kind="ExternalInput"),
        "v": nc.dram_tensor("Wv", [E, E], BF16, kind="ExternalInput"),
        "o": nc.dram_tensor("Wo", [E, E], BF16, kind="ExternalInput"),
    }
    dbo = nc.dram_tensor("bo", [E], FP32, kind="ExternalInput")
    dout = nc.dram_tensor("out", [N, E], FP32, kind="ExternalOutput")
    drs = nc.dram_tensor("r_scratch", [H * N], FP32)

    with tile.TileContext(nc) as tc, ExitStack() as ctx:
        _body(ctx, tc, dQ, dK, dV, dW, dbo, dout, drs)
    nc.finalize()
    return nc


def _body(ctx, tc, dQ, dK, dV, dW, dbo, dout, drs):
    nc = tc.nc
    dma = nc.sync.dma_start

    const = ctx.enter_context(tc.tile_pool(name="const", bufs=1))
    # 12 x [128, N] bf16 slots reused across phases:
    #   stage 0: Q^T (big_0..3) / K^T (big_4..7) / V^T (big_8..11)
    #   attn: oT (big_8..11), oTn (big_0..3)
    big = ctx.enter_context(tc.tile_pool(name="big", bufs=1))
    proj = ctx.enter_context(tc.tile_pool(name="proj", bufs=1))
    # PSUM budget (8 banks of [128,512] f32):
    #   s2 ([128,1024] f32, bufs=3) -> 6 banks: attention scores; proj and
    #       out-proj fillers ride the same ring with [128,512] tiles
    #   o2e/o2o ([65,512] f32 PV accum, bufs=1) -> 2 banks
    psum = ctx.enter_context(tc.tile_pool(name="psum", bufs=1, space="PSUM"))
    stage = ctx.enter_context(tc.tile_pool(name="stage", bufs=4))
    p2pool = ctx.enter_context(tc.tile_pool(name="p2pool", bufs=3))

    # bias replicated across all partitions (for the direct-orientation
    # output projection the bias varies along the free dim)
    bo_full = const.tile([P, E], FP32, name="bo_full", tag="bo_full")
    dma(out=bo_full, in_=bass.AP(tensor=dbo, offset=0, ap=[[0, P], [1, E]]))

    l1 = const.tile([1, H * N], FP32, name="l1", tag="l1")
    ltmp = const.tile([P, 2 * N // P], FP32, name="ltmp", tag="ltmp")
    ones_f32 = const.tile([1, P], FP32, name="ones_f32", tag="ones_f32")
    nc.gpsimd.memset(ones_f32, 1.0)

    # ---- stage 0: all transposes via DMA xbar, straight from DRAM ----
    wt = {}
    for wname in ("q", "k", "v", "o"):
        wt[wname] = [const.tile([P, E], BF16, name=f"w{wname}T_{c}",
                                tag=f"w{wname}T_{c}") for c in range(ET)]

    def emit_wT(wname):
        # wt[c][i, o] = W[o, c*128+i]
        for c in range(ET):
            dma(out=wt[wname][c], in_=dW[wname][:, c * P:(c + 1) * P],
                transpose=True)

    xT = {}
    slot = {"K": 4, "Q": 0, "V": 8}
    for xname in ("K", "Q", "V"):
        xT[xname] = [big.tile([P, N], BF16, name=f"{xname}T_{et}",
                              tag=f"big_{slot[xname] + et}")
                     for et in range(ET)]

    def emit_xT(xname, dX, et):
        # xT[et] = X[:, et*128:(et+1)*128]^T   ([2048,128] -> [128,2048])
        dma(out=xT[xname][et], in_=dX[:, et * P:(et + 1) * P], transpose=True)

    # ---- projections ----
    qT = [proj.tile([P, N], BF16, name=f"qT_{m}", tag=f"qT_{m}")
          for m in range(ET)]
    kT = [proj.tile([P, N], BF16, name=f"kT_{m}", tag=f"kT_{m}")
          for m in range(ET)]
    v_aug = [proj.tile([P, H * HV], BF16, name=f"vaug_{nt}",
                       tag=f"vaug_{nt}") for nt in range(NT)]

    def emit_qk_proj(m, c, names=("q", "k")):
        """qT[m] and/or kT[m], n-chunk c. Rides the s2 PSUM ring (o2e/o2o
        are live PV accumulators once attention starts)."""
        for pname, outs in (("q", qT), ("k", kT)):
            if pname not in names:
                continue
            xtiles = xT[pname.upper()]
            ps = psum.tile([P, 512], FP32, name="pp", tag="s2", bufs=3)
            for et in range(ET):
                nc.tensor.matmul(
                    ps,
                    lhsT=wt[pname][et][:, m * P:(m + 1) * P],
                    rhs=xtiles[et][:, c * 512:(c + 1) * 512],
                    start=(et == 0), stop=(et == ET - 1))
            nc.vector.tensor_copy(outs[m][:, c * 512:(c + 1) * 512], ps)

    def emit_v_proj(nt):
        ps = psum.tile([P, 512], FP32, name="pp", tag="s2", bufs=3)
        for et in range(ET):
            nc.tensor.matmul(
                ps,
                lhsT=xT["V"][et][:, nt * P:(nt + 1) * P],
                rhs=wt["v"][et],
                start=(et == 0), stop=(et == ET - 1))
        va = v_aug[nt].rearrange("p (h c) -> p h c", c=HV)
        nc.vector.tensor_copy(
            va[:, :, 0:D], ps.rearrange("p (h d) -> p h d", d=D))
        nc.gpsimd.memset(va[:, :, D:HV], 1.0)

    # Ordering minimizes time-to-first-exp: transposes are DMA-only, so the
    # PE ramp is only what chunk (0,0)'s first kts need: kT[0] (scores walk
    # the full row), qT[0] chunk 0, v_aug 0-1 (PV lags exp by one kt).
    # Everything else drips in as fillers under the exp wall.
    emit_wT("k")
    for et in range(ET):
        emit_xT("K", dK, et)
    for c in range(NQC):
        emit_qk_proj(0, c, names=("k",))
    emit_wT("q")
    for et in range(ET):
        emit_xT("Q", dQ, et)
    emit_qk_proj(0, 0, names=("q",))
    emit_wT("v")
    for et in range(ET):
        emit_xT("V", dV, et)
    for nt in range(4):
        emit_v_proj(nt)
    emit_wT("o")

    # Deferred stage-0 work, drip-fed during head-pair 0's chunks so the PE
    # never bursts ahead of the 1148ns/kt ACT exp rate. Units are single
    # projections (~1.1us PE each) so no pop stalls the exp stream long.
    # Chunk (0,0) pops every kt<=13: v_aug 4..15 in kt order (v_j lands ~3
    # kts before PV kt_j consumes it), then qT[0] c1/c2. Later hp0 chunks
    # pop at kt%2==1: qT[0] c3, then kT[m]/qT[m] m=1..3 (each needed first
    # by chunk (m, 0)).
    filler = deque()
    for nt in range(4, NT):
        filler.append(lambda nt=nt: emit_v_proj(nt))
    for c in range(1, ET):
        filler.append(lambda c=c: emit_qk_proj(0, c, names=("q",)))
    for m in range(1, ET):
        for c in range(ET):
            filler.append(lambda m=m, c=c: emit_qk_proj(m, c, names=("k",)))
        for c in range(ET):
            filler.append(lambda m=m, c=c: emit_qk_proj(m, c, names=("q",)))

    # ---- attention ----
    oT = [big.tile([P, N], BF16, name=f"oT_{m}", tag=f"big_{8 + m}")
          for m in range(ET)]
    oTn = [big.tile([P, N], BF16, name=f"oTn_{m}", tag=f"big_{m}")
           for m in range(ET)]
    y_fill = deque()  # out-projection units, dripped during hp3's chunks

    def attn_chunk(hp, qc):
        """Both heads of pair hp, q-chunk qc. Row-tiled scores (head-even on
        PE rows 0-63, head-odd on 64-127) stream into one [128,1024] PSUM
        tile -> one [128,1024] exp per kt on ScalarE. PV lagged by one."""
        he, ho = 2 * hp, 2 * hp + 1
        o2 = {0: psum.tile([HV, QC], FP32, name="o2e", tag="o2e", bufs=1),
              1: psum.tile([HV, QC], FP32, name="o2o", tag="o2o", bufs=1)}
        rq = {0: qT[hp][0:64, qc * QC:(qc + 1) * QC],
              1: qT[hp][64:128, qc * QC:(qc + 1) * QC]}
        va = [v_aug[kt].rearrange("p (h c) -> p h c", c=HV)
              for kt in range(NT)]
        prev = None  # (p2, kt) pending PV

        def emit_pv(p2, kt):
            for h2 in (0, 1):
                nc.tensor.matmul(
                    o2[h2], lhsT=va[kt][:, 2 * hp + h2, :],
                    rhs=p2[:, h2 * QC:(h2 + 1) * QC],
                    start=(kt == 0), stop=(kt == NT - 1),
                    skip_group_check=True)

        for kt in range(NT):
            s2 = psum.tile([P, 2 * QC], FP32, name="s2", tag="s2", bufs=3)
            for h2 in (0, 1):
                nc.tensor.matmul(
                    s2[:, h2 * QC:(h2 + 1) * QC],
                    lhsT=kT[hp][h2 * 64:h2 * 64 + 64, kt * P:(kt + 1) * P],
                    rhs=rq[h2], start=True, stop=True)
            if prev is not None:
                emit_pv(*prev)
            p2 = p2pool.tile([P, 2 * QC], BF16, name="p2", tag="p2")
            nc.scalar.activation(p2, s2, AF.Exp, scale=LN2)
            prev = (p2, kt)
            if hp == 0 and filler and (
                    (qc == 0 and kt <= 13) or (qc >= 1 and kt % 2 == 1)):
                filler.popleft()()
            # y units pop mid-chunk (kt 7..13): their oTn[3] input comes
            # from the previous chunk's norm, ~4us after that chunk ended
            if hp == 3 and y_fill and 7 <= kt <= 13 and kt % 2 == 1:
                y_fill.popleft()()
        emit_pv(*prev)
        # drains (DVE; ScalarE stays on exp)
        nc.vector.tensor_copy(oT[hp][0:64, qc * QC:(qc + 1) * QC],
                              o2[0][0:D, :])
        nc.vector.tensor_copy(oT[hp][64:128, qc * QC:(qc + 1) * QC],
                              o2[1][0:D, :])
        if hp < 3:
            nc.vector.tensor_copy(
                l1[0:1, he * N + qc * QC:he * N + (qc + 1) * QC],
                o2[0][D:HV, :])
            nc.vector.tensor_copy(
                l1[0:1, ho * N + qc * QC:ho * N + (qc + 1) * QC],
                o2[1][D:HV, :])
        else:
            # hp3 normalizes per-qc with no DRAM round-trip: reciprocal of
            # the denominator row straight from PSUM, broadcast across
            # partitions via a ones-column PE matmul, multiply on DVE.
            rb_ps = psum.tile([P, QC], FP32, name="rb_ps", tag="s2", bufs=3)
            for h2 in range(2):
                r_row = stage.tile([1, QC], FP32, name="r_row",
                                   tag="r_row", bufs=2)
                nc.vector.reciprocal(r_row, o2[h2][D:HV, :])
                nc.tensor.matmul(
                    rb_ps[h2 * 64:(h2 + 1) * 64, :],
                    lhsT=ones_f32[0:1, 0:64], rhs=r_row,
                    start=True, stop=True, skip_group_check=True)
            nc.vector.tensor_tensor(
                oTn[3][:, qc * QC:(qc + 1) * QC],
                oT[3][:, qc * QC:(qc + 1) * QC],
                rb_ps, ALU.mult)

    def emit_norm_hp(hp):
        """Whole-hp normalization (hp 0-2): one reciprocal round-trip and a
        single [128,2048] multiply, issued after the hp's 4th chunk."""
        for h2 in range(2):
            h = hp * 2 + h2
            dma(out=ltmp[:, h2 * 16:h2 * 16 + 16],
                in_=l1[0:1, h * N:(h + 1) * N])
        nc.vector.reciprocal(ltmp, ltmp)
        for h2 in range(2):
            h = hp * 2 + h2
            dma(out=drs[h * N:(h + 1) * N], in_=ltmp[:, h2 * 16:h2 * 16 + 16])
        rb = stage.tile([P, N], FP32, name="rb", tag="rb", bufs=1)
        for h2 in range(2):
            h = hp * 2 + h2
            bsrc = bass.AP(tensor=drs, offset=h * N, ap=[[0, 64], [1, N]])
            dma(out=rb[h2 * 64:(h2 + 1) * 64, :], in_=bsrc)
        nc.vector.tensor_tensor(oTn[hp], oT[hp], rb, ALU.mult)

    # ---- output projection, direct orientation: y[nt] = O^T-block^T @ WoT
    def y_unit(nt):
        ps = psum.tile([P, 512], FP32, name="ps_o", tag="s2", bufs=3)
        for et in range(ET):
            nc.tensor.matmul(
                ps,
                lhsT=oTn[et][:, nt * P:(nt + 1) * P],
                rhs=wt["o"][et],
                start=(et == 0), stop=(et == ET - 1))
        y_sb = stage.tile([P, E], FP32, name="y_sb", tag="y_sb", bufs=2)
        nc.vector.tensor_tensor(y_sb, ps, bo_full, ALU.add)
        dma(out=dout[nt * P:(nt + 1) * P, :], in_=y_sb)

    for hp in range(ET):
        for qc in range(NQC):
            attn_chunk(hp, qc)
            if hp == 3:
                for nt in range(4 * qc, 4 * qc + 4):
                    y_fill.append(lambda nt=nt: y_unit(nt))
        if hp < 3:
            emit_norm_hp(hp)
        if hp == 0:
            while filler:  # safety: must drain before hp1 needs qT/kT[1]
                filler.popleft()()
    while y_fill:
        y_fill.popleft()()


_CACHE = {}


def _get_nc() -> bass.Bass:
    if "nc" not in _CACHE:
        _CACHE["nc"] = _build()
    return _CACHE["nc"]


def _prep_core_inputs(inputs: dict) -> list:
    """Host-side prep: bf16 casts + tau/log2e folding into Wq. Returns the
    per-core input maps."""
    import ml_dtypes

    bf16 = ml_dtypes.bfloat16
    tau = float(np.asarray(inputs["tau"]))
    Q = np.asarray(inputs["Q"], dtype=np.float32)
    K = np.asarray(inputs["K"], dtype=np.float32)
    V = np.asarray(inputs["V"], dtype=np.float32)
    Wq = (np.asarray(inputs["Wq"], dtype=np.float32) * (LOG2E / tau)).astype(bf16)
    Wk = np.asarray(inputs["Wk"], dtype=np.float32).astype(bf16)
    Wv = np.asarray(inputs["Wv"], dtype=np.float32).astype(bf16)
    Wo = np.asarray(inputs["Wo"], dtype=np.float32).astype(bf16)
    bo = np.ascontiguousarray(np.asarray(inputs["bo"], dtype=np.float32))
    Qb = Q.astype(bf16)
    Kb = K.astype(bf16)
    Vb = V.astype(bf16)
    in_maps = []
    for b in range(NCORES):
        in_maps.append({
            "Q": np.ascontiguousarray(Qb[b]),
            "K": np.ascontiguousarray(Kb[b]),
            "V": np.ascontiguousarray(Vb[b]),
            "Wq": Wq, "Wk": Wk, "Wv": Wv, "Wo": Wo, "bo": bo,
        })
    return in_maps


def _run(inputs: dict, trace: bool = False):
    """Returns (output [B,N,E] fp32, BassKernelResults)."""
    from concourse.bass_utils import run_bass_kernel_spmd

    mask = inputs.get("attn_mask")
    if mask is not None and not np.all(np.asarray(mask) != 0):
        # Fallback (never hit for the spec'd all-ones mask): host math.
        return _host_reference(
            np.asarray(inputs["Q"], dtype=np.float32),
            np.asarray(inputs["K"], dtype=np.float32),
            np.asarray(inputs["V"], dtype=np.float32),
            np.asarray(mask),
            np.asarray(inputs["Wq"], dtype=np.float32),
            np.asarray(inputs["Wk"], dtype=np.float32),
            np.asarray(inputs["Wv"], dtype=np.float32),
            np.asarray(inputs["Wo"], dtype=np.float32),
            np.asarray(inputs["bo"], dtype=np.float32),
            float(np.asarray(inputs["tau"]))), None

    nc = _get_nc()
    in_maps = _prep_core_inputs(inputs)
    res = run_bass_kernel_spmd(nc, in_maps, list(range(NCORES)), trace=trace)
    out = np.stack([np.asarray(res.results[b]["out"]) for b in range(NCORES)])
    return out.astype(np.float32), res


def _host_reference(Q, K, V, mask, Wq, Wk, Wv, Wo, bo, tau):
    b, n, _ = Q.shape
    q = (Q @ Wq.T).reshape(b, n, H, D).transpose(0, 2, 1, 3)
    k = (K @ Wk.T).reshape(b, n, H, D).transpose(0, 2, 1, 3)
    v = (V @ Wv.T).reshape(b, n, H, D).transpose(0, 2, 1, 3)
    s = np.einsum("bhnd,bhmd->bhnm", q, k) / tau
    s = np.where(mask == 0, -np.inf, s)
    s = s - s.max(axis=-1, keepdims=True)
    e = np.exp(s)
    a = e / e.sum(axis=-1, keepdims=True)
    o = np.einsum("bhnm,bhmd->bhnd", a, v)
    o = o.transpose(0, 2, 1, 3).reshape(b, n, H * D)
    return (o @ Wo.T + bo).astype(np.float32)


def kernel(**inputs) -> np.ndarray:
    out, _ = _run(inputs, trace=False)
    return out


# revision 24
# speedup vs baseline: 1.0326x; 1.0326x over previous
"""MultiHeadSelfAttention Trainium2 Bass kernel (v4).

Shapes (hardcoded): B=8, N=2048, E=512, H=8 heads, D=64 head dim.
Sharding: data-parallel over batch -> one batch item per NeuronCore (8 cores),
no collectives needed.

v4 design (vs v2 at ~420us measured):
  - Inputs are cast to bf16 on the HOST (the kernel computed in bf16 SBUF
    tiles anyway), so ALL stage-0 transposes (Q/K/V/W -> column-major) run
    as DMA xbar transposes straight from DRAM -- one [2048,128]->[128,2048]
    instruction per (input, e-tile): zero PE/DVE/ACT cost where v2 spent
    78us PE (fp32 transposes) + 84us DVE + 26us ACT (copy casts).
  - Output projection runs in direct (non-transposed) orientation:
    y[n-tile] = sum_et oTn[et][:, n-block]^T @ WoT[et] -- the attention
    output O^T is the lhsT, so no de-transpose/cast tail at all. Units are
    dripped late (kt>=9) into hp3's chunks so the in-order PE stream never
    blocks on the per-chunk norm DMA chain.
  - Normalization is batched per-hp for hp0-2 (one [128,2048] multiply +
    6 DMAs instead of 4x per-qc chains); hp3 keeps per-qc norm so y units
    unlock chunk by chunk.
  - exp offload history: DVE tensor_tensor pow fails the TRN2 ISA engine
    check; GpSimd pow compiles but runs as a DSP software loop (~169us per
    [128,1024] tile, measured) -- both dead ends. All 256 exps stay on
    ScalarE (the kernel's ~294us floor); everything else hides under them.
    Scores arrive pre-scaled by log2e/tau (folded into Wq on the host), so
    the ACTIVATE uses scale=LN2.
  - Attention core unchanged from v2: scores 2 heads/kt via PE row tiling,
    one [128,1024] exp per kt, PV lagged one kt, v_aug ones-column gives
    softmax denominators for free.
"""

import sys

for _p in ("/opt/trn_rl_repo",):
    if _p not in sys.path:
        sys.path.insert(0, _p)

import numpy as np
from collections import deque
from contextlib import ExitStack

import concourse.bass as bass
import concourse.bacc as bacc
import concourse.mybir as mybir
import concourse.tile as tile

B, N, E = 8, 2048, 512
H, D = 8, 64
P = 128          # partitions
ET = E // P      # 4 e-tiles
NT = N // P      # 16 n-tiles
QC = 512         # q chunk in attention
NQC = N // QC    # 4
HV = 65          # head dim + ones column
FP32 = mybir.dt.float32
BF16 = mybir.dt.bfloat16
NCORES = 8

AF = mybir.ActivationFunctionType
ALU = mybir.AluOpType
LOG2E = 1.4426950408889634
LN2 = 0.6931471805599453


def _build() -> bass.Bass:
    nc = bacc.Bacc(trn_type="TRN2")

    dQ = nc.dram_tensor("Q", [N, E], BF16, kind="ExternalInput")
    dK = nc.dram_tensor("K", [N, E], BF16, kind="ExternalInput")
    dV = nc.dram_tensor("V", [N, E], BF16, kind="ExternalInput")
    dW = {
        "q": nc.dram_tensor("Wq", [E, E], BF16, kind="ExternalInput"),
        "k": nc.dram_tensor("Wk", [E, E], BF16, kind="ExternalInput"),
        "v": nc.dram_tensor("Wv", [E, E], BF16, kind="ExternalInput"),
        "o": nc.dram_tensor("Wo", [E, E], BF16, kind="ExternalInput"),
    }
    dbo = nc.dram_tensor("bo", [E], FP32, kind="ExternalInput")
    dout = nc.dram_tensor("out", [N, E], FP32, kind="ExternalOutput")
    drs = nc.dram_tensor("r_scratch", [H * N], FP32)

    with tile.TileContext(nc) as tc, ExitStack() as ctx:
        _body(ctx, tc, dQ, dK, dV, dW, dbo, dout, drs)
    nc.finalize()
    return nc


def _body(ctx, tc, dQ, dK, dV, dW, dbo, dout, drs):
    nc = tc.nc
    dma = nc.sync.dma_start

    const = ctx.enter_context(tc.tile_pool(name="const", bufs=1))
    # 12 x [128, N] bf16 slots reused across phases:
    #   stage 0: Q^T (big_0..3) / K^T (big_4..7) / V^T (big_8..11)
    #   attn: oT (big_8..11), oTn (big_0..3)
    big = ctx.enter_context(tc.tile_pool(name="big", bufs=1))
    proj = ctx.enter_context(tc.tile_pool(name="proj", bufs=1))
    # PSUM budget (8 banks of [128,512] f32):
    #   s2 ([128,1024] f32, bufs=3) -> 6 banks: attention scores; proj and
    #       out-proj fillers ride the same ring with [128,512] tiles
    #   o2e/o2o ([65,512] f32 PV accum, bufs=1) -> 2 banks
    psum = ctx.enter_context(tc.tile_pool(name="psum", bufs=1, space="PSUM"))
    stage = ctx.enter_context(tc.tile_pool(name="stage", bufs=4))
    p2pool = ctx.enter_context(tc.tile_pool(name="p2pool", bufs=3))

    # bias replicated across all partitions (for the direct-orientation
    # output projection the bias varies along the free dim)
    bo_full = const.tile([P, E], FP32, name="bo_full", tag="bo_full")
    dma(out=bo_full, in_=bass.AP(tensor=dbo, offset=0, ap=[[0, P], [1, E]]))

    l1 = const.tile([1, H * N], FP32, name="l1", tag="l1")
    ltmp = const.tile([P, 2 * N // P], FP32, name="ltmp", tag="ltmp")
    # all-ones [128, 64] so a [1, 64] slice exists at any base partition
    # (matmul requires lhsT and rhs to share their base partition)
    ones_bf = const.tile([P, 64], BF16, name="ones_bf", tag="ones_bf")
    nc.gpsimd.memset(ones_bf, 1.0)

    # ---- stage 0: all transposes via DMA xbar, straight from DRAM ----
    wt = {}
    for wname in ("q", "k", "v", "o"):
        wt[wname] = [const.tile([P, E], BF16, name=f"w{wname}T_{c}",
                                tag=f"w{wname}T_{c}") for c in range(ET)]

    def emit_wT(wname):
        # wt[c][i, o] = W[o, c*128+i]
        for c in range(ET):
            dma(out=wt[wname][c], in_=dW[wname][:, c * P:(c + 1) * P],
                transpose=True)

    xT = {}
    slot = {"K": 4, "Q": 0, "V": 8}
    for xname in ("K", "Q", "V"):
        xT[xname] = [big.tile([P, N], BF16, name=f"{xname}T_{et}",
                              tag=f"big_{slot[xname] + et}")
                     for et in range(ET)]

    def emit_xT(xname, dX, et):
        # xT[et] = X[:, et*128:(et+1)*128]^T   ([2048,128] -> [128,2048])
        dma(out=xT[xname][et], in_=dX[:, et * P:(et + 1) * P], transpose=True)

    # ---- projections ----
    qT = [proj.tile([P, N], BF16, name=f"qT_{m}", tag=f"qT_{m}")
          for m in range(ET)]
    kT = [proj.tile([P, N], BF16, name=f"kT_{m}", tag=f"kT_{m}")
          for m in range(ET)]
    v_aug = [proj.tile([P, H * HV], BF16, name=f"vaug_{nt}",
                       tag=f"vaug_{nt}") for nt in range(NT)]

    def emit_qk_proj(m, c, names=("q", "k")):
        """qT[m] and/or kT[m], n-chunk c. Rides the s2 PSUM ring (o2e/o2o
        are live PV accumulators once attention starts)."""
        for pname, outs in (("q", qT), ("k", kT)):
            if pname not in names:
                continue
            xtiles = xT[pname.upper()]
            ps = psum.tile([P, 512], FP32, name="pp", tag="s2", bufs=3)
            for et in range(ET):
                nc.tensor.matmul(
                    ps,
                    lhsT=wt[pname][et][:, m * P:(m + 1) * P],
                    rhs=xtiles[et][:, c * 512:(c + 1) * 512],
                    start=(et == 0), stop=(et == ET - 1))
            nc.vector.tensor_copy(outs[m][:, c * 512:(c + 1) * 512], ps)

    def emit_v_proj(nt):
        ps = psum.tile([P, 512], FP32, name="pp", tag="s2", bufs=3)
        for et in range(ET):
            nc.tensor.matmul(
                ps,
                lhsT=xT["V"][et][:, nt * P:(nt + 1) * P],
                rhs=wt["v"][et],
                start=(et == 0), stop=(et == ET - 1))
        va = v_aug[nt].rearrange("p (h c) -> p h c", c=HV)
        nc.vector.tensor_copy(
            va[:, :, 0:D], ps.rearrange("p (h d) -> p h d", d=D))
        nc.gpsimd.memset(va[:, :, D:HV], 1.0)

    # Ordering minimizes time-to-first-exp: transposes are DMA-only, so the
    # PE ramp is only what chunk (0,0)'s first kts need: kT[0] (scores walk
    # the full row), qT[0] chunk 0, v_aug 0-1 (PV lags exp by one kt).
    # Everything else drips in as fillers under the exp wall.
    emit_wT("k")
    for et in range(ET):
        emit_xT("K", dK, et)
    for c in range(NQC):
        emit_qk_proj(0, c, names=("k",))
    emit_wT("q")
    for et in range(ET):
        emit_xT("Q", dQ, et)
    emit_qk_proj(0, 0, names=("q",))
    emit_wT("v")
    for et in range(ET):
        emit_xT("V", dV, et)
    for nt in range(4):
        emit_v_proj(nt)
    emit_wT("o")

    # Deferred stage-0 work, drip-fed under the exp wall. Units are single
    # projections (~1.1us PE each) so no pop stalls the exp stream long.
    # fillers[h] holds units that must complete before head-pair h starts:
    # chunk (0,0) pops every kt<=13 (v_aug 4..15 in kt order -- v_j lands
    # ~3 kts before PV kt_j consumes it -- then qT[0] c1/c2); remaining hp0
    # chunks pop fillers[1] at kt%4==1, hp1 pops fillers[2], hp2 fillers[3].
    fillers = {1: deque(), 2: deque(), 3: deque()}
    for nt in range(4, NT):
        fillers[1].append(lambda nt=nt: emit_v_proj(nt))
    for c in range(1, ET):
        fillers[1].append(lambda c=c: emit_qk_proj(0, c, names=("q",)))
    for m in range(1, ET):
        for c in range(ET):
            fillers[m].append(lambda m=m, c=c: emit_qk_proj(m, c, names=("k",)))
        for c in range(ET):
            fillers[m].append(lambda m=m, c=c: emit_qk_proj(m, c, names=("q",)))

    # ---- attention ----
    oT = [big.tile([P, N], BF16, name=f"oT_{m}", tag=f"big_{8 + m}")
          for m in range(3)]  # hp3 drains via d65 instead
    oTn = [big.tile([P, N], BF16, name=f"oTn_{m}", tag=f"big_{m}")
           for m in range(ET)]
    y_fill = deque()  # out-projection units, dripped during hp3's chunks

    def attn_chunk(hp, qc):
        """Both heads of pair hp, q-chunk qc. Row-tiled scores (head-even on
        PE rows 0-63, head-odd on 64-127) stream into one [128,1024] PSUM
        tile -> one [128,1024] exp per kt on ScalarE. PV lagged by one."""
        he, ho = 2 * hp, 2 * hp + 1
        o2 = {0: psum.tile([HV, QC], FP32, name="o2e", tag="o2e", bufs=1),
              1: psum.tile([HV, QC], FP32, name="o2o", tag="o2o", bufs=1)}
        rq = {0: qT[hp][0:64, qc * QC:(qc + 1) * QC],
              1: qT[hp][64:128, qc * QC:(qc + 1) * QC]}
        va = [v_aug[kt].rearrange("p (h c) -> p h c", c=HV)
              for kt in range(NT)]
        prev = None  # (p2, kt) pending PV

        def emit_pv(p2, kt):
            for h2 in (0, 1):
                nc.tensor.matmul(
                    o2[h2], lhsT=va[kt][:, 2 * hp + h2, :],
                    rhs=p2[:, h2 * QC:(h2 + 1) * QC],
                    start=(kt == 0), stop=(kt == NT - 1),
                    skip_group_check=True)

        for kt in range(NT):
            s2 = psum.tile([P, 2 * QC], FP32, name="s2", tag="s2", bufs=3)
            for h2 in (0, 1):
                nc.tensor.matmul(
                    s2[:, h2 * QC:(h2 + 1) * QC],
                    lhsT=kT[hp][h2 * 64:h2 * 64 + 64, kt * P:(kt + 1) * P],
                    rhs=rq[h2], start=True, stop=True)
            if prev is not None:
                emit_pv(*prev)
            p2 = p2pool.tile([P, 2 * QC], BF16, name="p2", tag="p2")
            nc.scalar.activation(p2, s2, AF.Exp, scale=LN2)
            prev = (p2, kt)
            if hp == 0 and qc == 0 and fillers[1] and kt <= 13:
                fillers[1].popleft()()
            elif hp == 0 and qc >= 1 and fillers[1] and kt % 4 == 1:
                fillers[1].popleft()()
            elif hp in (1, 2) and fillers[hp + 1] and kt % 4 == 1:
                fillers[hp + 1].popleft()()
            # y units pop mid-chunk (kt 7..13): their oTn[3] input comes
            # from the previous chunk's norm, ~3us after that chunk ended
            if hp == 3 and y_fill and 7 <= kt <= 13 and kt % 2 == 1:
                y_fill.popleft()()
        emit_pv(*prev)
        if hp < 3:
            # drains (DVE; ScalarE stays on exp)
            nc.vector.tensor_copy(oT[hp][0:64, qc * QC:(qc + 1) * QC],
                                  o2[0][0:D, :])
            nc.vector.tensor_copy(oT[hp][64:128, qc * QC:(qc + 1) * QC],
                                  o2[1][0:D, :])
            nc.vector.tensor_copy(
                l1[0:1, he * N + qc * QC:he * N + (qc + 1) * QC],
                o2[0][D:HV, :])
            nc.vector.tensor_copy(
                l1[0:1, ho * N + qc * QC:ho * N + (qc + 1) * QC],
                o2[1][D:HV, :])
        else:
            # hp3 normalizes per-qc with no DRAM round-trip and no
            # single-partition DVE ops (those run at ~6.5ns/elem): drain
            # o2 rows 0..64 to SBUF bf16 in one parallel copy, broadcast
            # the denominator row via a bf16 ones-matmul (~213ns), then a
            # single DVE divide per head.
            rb_ps = psum.tile([P, QC], FP32, name="rb_ps", tag="s2", bufs=3)
            d65 = [stage.tile([HV, QC], BF16, name=f"d65_{h2}",
                              tag=f"d65_{h2}", bufs=2) for h2 in range(2)]
            for h2 in range(2):
                nc.vector.tensor_copy(d65[h2], o2[h2])
                nc.tensor.matmul(
                    rb_ps[h2 * 64:(h2 + 1) * 64, :],
                    lhsT=ones_bf[D:HV, :], rhs=d65[h2][D:HV, :],
                    start=True, stop=True, skip_group_check=True)
            for h2 in range(2):
                # DVE has no divide in the TRN2 ISA: reciprocal on the
                # broadcast (so it runs 64 partitions wide), then multiply
                rn = stage.tile([64, QC], FP32, name="rn", tag="rn", bufs=2)
                nc.vector.reciprocal(rn, rb_ps[h2 * 64:(h2 + 1) * 64, :])
                nc.vector.tensor_tensor(
                    oTn[3][h2 * 64:(h2 + 1) * 64, qc * QC:(qc + 1) * QC],
                    d65[h2][0:D, :], rn, ALU.mult)

    def emit_norm_hp(hp):
        """Whole-hp normalization (hp 0-2): one reciprocal round-trip and a
        single [128,2048] multiply, issued after the hp's 4th chunk."""
        for h2 in range(2):
            h = hp * 2 + h2
            dma(out=ltmp[:, h2 * 16:h2 * 16 + 16],
                in_=l1[0:1, h * N:(h + 1) * N])
        nc.vector.reciprocal(ltmp, ltmp)
        for h2 in range(2):
            h = hp * 2 + h2
            dma(out=drs[h * N:(h + 1) * N], in_=ltmp[:, h2 * 16:h2 * 16 + 16])
        rb = stage.tile([P, N], FP32, name="rb", tag="rb", bufs=1)
        for h2 in range(2):
            h = hp * 2 + h2
            bsrc = bass.AP(tensor=drs, offset=h * N, ap=[[0, 64], [1, N]])
            dma(out=rb[h2 * 64:(h2 + 1) * 64, :], in_=bsrc)
        nc.vector.tensor_tensor(oTn[hp], oT[hp], rb, ALU.mult)

    # ---- output projection, direct orientation: y[nt] = O^T-block^T @ WoT
    def y_unit(nt):
        ps = psum.tile([P, 512], FP32, name="ps_o", tag="s2", bufs=3)
        for et in range(ET):
            nc.tensor.matmul(
                ps,
                lhsT=oTn[et][:, nt * P:(nt + 1) * P],
                rhs=wt["o"][et],
                start=(et == 0), stop=(et == ET - 1))
        y_sb = stage.tile([P, E], FP32, name="y_sb", tag="y_sb", bufs=2)
        nc.vector.tensor_tensor(y_sb, ps, bo_full, ALU.add)
        dma(out=dout[nt * P:(nt + 1) * P, :], in_=y_sb)

    for hp in range(ET):
        for qc in range(NQC):
            attn_chunk(hp, qc)
            if hp == 3:
                for nt in range(4 * qc, 4 * qc + 4):
                    y_fill.append(lambda nt=nt: y_unit(nt))
        if hp < 3:
            emit_norm_hp(hp)
            # safety: hp+1's qT/kT must exist before its first chunk
            while fillers[hp + 1]:
                fillers[hp + 1].popleft()()
    while y_fill:
        y_fill.popleft()()


_CACHE = {}


def _get_nc() -> bass.Bass:
    if "nc" not in _CACHE:
        _CACHE["nc"] = _build()
    return _CACHE["nc"]


def _prep_core_inputs(inputs: dict) -> list:
    """Host-side prep: bf16 casts + tau/log2e folding into Wq. Returns the
    per-core input maps."""
    import ml_dtypes

    bf16 = ml_dtypes.bfloat16
    tau = float(np.asarray(inputs["tau"]))
    Q = np.asarray(inputs["Q"], dtype=np.float32)
    K = np.asarray(inputs["K"], dtype=np.float32)
    V = np.asarray(inputs["V"], dtype=np.float32)
    Wq = (np.asarray(inputs["Wq"], dtype=np.float32) * (LOG2E / tau)).astype(bf16)
    Wk = np.asarray(inputs["Wk"], dtype=np.float32).astype(bf16)
    Wv = np.asarray(inputs["Wv"], dtype=np.float32).astype(bf16)
    Wo = np.asarray(inputs["Wo"], dtype=np.float32).astype(bf16)
    bo = np.ascontiguousarray(np.asarray(inputs["bo"], dtype=np.float32))
    Qb = Q.astype(bf16)
    Kb = K.astype(bf16)
    Vb = V.astype(bf16)
    in_maps = []
    for b in range(NCORES):
        in_maps.append({
            "Q": np.ascontiguousarray(Qb[b]),
            "K": np.ascontiguousarray(Kb[b]),
            "V": np.ascontiguousarray(Vb[b]),
            "Wq": Wq, "Wk": Wk, "Wv": Wv, "Wo": Wo, "bo": bo,
        })
    return in_maps


def _run(inputs: dict, trace: bool = False):
    """Returns (output [B,N,E] fp32, BassKernelResults)."""
    from concourse.bass_utils import run_bass_kernel_spmd

    mask = inputs.get("attn_mask")
    if mask is not None and not np.all(np.asarray(mask) != 0):
        # Fallback (never hit for the spec'd all-ones mask): host math.
        return _host_reference(
            np.asarray(inputs["Q"], dtype=np.float32),
            np.asarray(inputs["K"], dtype=np.float32),
            np.asarray(inputs["V"], dtype=np.float32),
            np.asarray(mask),
            np.asarray(inputs["Wq"], dtype=np.float32),
            np.asarray(inputs["Wk"], dtype=np.float32),
            np.asarray(inputs["Wv"], dtype=np.float32),
            np.asarray(inputs["Wo"], dtype=np.float32),
            np.asarray(inputs["bo"], dtype=np.float32),
            float(np.asarray(inputs["tau"]))), None

    nc = _get_nc()
    in_maps = _prep_core_inputs(inputs)
    res = run_bass_kernel_spmd(nc, in_maps, list(range(NCORES)), trace=trace)
    out = np.stack([np.asarray(res.results[b]["out"]) for b in range(NCORES)])
    return out.astype(np.float32), res


def _host_reference(Q, K, V, mask, Wq, Wk, Wv, Wo, bo, tau):
    b, n, _ = Q.shape
    q = (Q @ Wq.T).reshape(b, n, H, D).transpose(0, 2, 1, 3)
    k = (K @ Wk.T).reshape(b, n, H, D).transpose(0, 2, 1, 3)
    v = (V @ Wv.T).reshape(b, n, H, D).transpose(0, 2, 1, 3)
    s = np.einsum("bhnd,bhmd->bhnm", q, k) / tau
    s = np.where(mask == 0, -np.inf, s)
    s = s - s.max(axis=-1, keepdims=True)
    e = np.exp(s)
    a = e / e.sum(axis=-1, keepdims=True)
    o = np.einsum("bhnm,bhmd->bhnd", a, v)
    o = o.transpose(0, 2, 1, 3).reshape(b, n, H * D)
    return (o @ Wo.T + bo).astype(np.float32)


def kernel(**inputs) -> np.ndarray:
    out, _ = _run(inputs, trace=False)
    return out
